# revision 35
# baseline (speedup 1.0000x reference)
"""Trainium2 Bass kernel for nn_AttentionBlock (GroupNorm + MHA + proj + residual).

Sharding: data-parallel over batch; 8 batches -> 8 NeuronCores, one batch each.

Per-core layout (c=512 channels, t=1024 spatial, H=8 heads, ch=64):
  - x, h kept as [c-on-partitions, t] (4 tiles of [128, 1024])
  - GroupNorm group-reduction done with two tiny mask matmuls on the PE
    (no cross-partition DMA shuffles)
  - q,k per head PAIR: qk[pi] = [p, {q,k}, t], partitions 0-63 = head 2pi,
    64-127 = head 2pi+1 (w_qkv rows permuted host-side; q pre-scaled by
    1/sqrt(ch) = 0.125, exact).  The two heads of a pair use different PE row
    groups, so their score matmuls run concurrently.
  - v computed directly transposed: vT[t, o_v] via matmul(lhsT=h, rhs=WvT), with
    a ones column per head -> the AV matmul also emits the softmax denominator Z
  - scores computed transposed: S^T[s, t] = k^T q, so exp(S^T) (ACT, psum->sbuf)
    feeds the AV matmul with s as the contraction dim; softmax skips the
    max-subtraction (scores are ~N(0,1), exp is safe in fp32)
  - Z normalization: evacuate AV psum, reshape Z across 128 partitions via a
    DRAM round trip, exact reciprocal on [128, 8], broadcast back, fused into
    the normalization multiply
  - proj + bias + residual fused in one scalar_tensor_tensor per output tile
All matmuls run in fp32r (same PE rate as bf16 here; ~1.5e-4 max rel err).
"""

import numpy as np

B, C, HW, T = 8, 512, 32, 1024
H, CH = 8, 64
G, GS = 32, 16
EPS = 1e-5
NCORES = 8

_CACHE = {}
TRACE = False  # test harness can set kernel.TRACE = True to get a profile


def _install_ntff_hook():
    import sys, types
    if 'antenv.axon_hooks' in sys.modules:
        return
    mod = types.ModuleType('antenv.axon_hooks')
    state = {'hook': None}
    mod.set_axon_ntff_profile_hook = lambda h: state.__setitem__('hook', h)
    mod.get_axon_ntff_profile_hook = lambda: state['hook']
    sys.modules['antenv.axon_hooks'] = mod
    import antenv
    antenv.axon_hooks = mod
    try:
        from trn_agent_boot.trn_boot import _ntff_profile_via_ctypes
        mod.set_axon_ntff_profile_hook(_ntff_profile_via_ctypes('/opt/axon/libaxon_pjrt.so'))
    except Exception:
        pass


def _patch_ldw_opt():
    """Let walrus dedup back-to-back LDWEIGHTS of the same stationary operand."""
    import concourse.bass_utils as bu
    if getattr(bu, "_ldw_opt_patched", False):
        return
    orig = bu.run_command

    def patched(argv, **kw):
        argv = ["--enable-ldw-opt=true" if a == "--enable-ldw-opt=false" else a
                for a in argv]
        return orig(argv, **kw)

    bu.run_command = patched
    bu._ldw_opt_patched = True


def _split_multi_waits(nc, max_waits=1):
    """This container's walrus supports only one sync wait per instruction; move
    extra waits onto same-engine no-ops inserted just before the instruction."""
    import concourse.mybir as mybir
    for f in nc.m.functions:
        for bb in f.blocks:
            insts = bb.instructions
            out = []
            changed = False
            for inst in insts:
                si = inst.sync_info
                waits = list(si.on_wait) if si is not None and si.on_wait else []
                if len(waits) > max_waits:
                    changed = True
                    for j, w in enumerate(waits[:-max_waits]):
                        out.append(mybir.InstNoOp(
                            name=f"{inst.name}-ws{j}",
                            sync_info=mybir.SyncInfo(on_wait=[w], on_update=[]),
                            bass_nofuse=True,
                            engine=inst.engine,
                        ))
                    inst.sync_info = mybir.SyncInfo(
                        on_wait=waits[-max_waits:],
                        on_update=list(si.on_update) if si.on_update else [],
                    )
                out.append(inst)
            if changed:
                bb.instructions = out


def _build_nc():
    import concourse.bass as bass
    import concourse.tile as tile
    import concourse.mybir as mybir

    f32 = mybir.dt.float32
    f32r = mybir.dt.float32r
    Alu = mybir.AluOpType
    Act = mybir.ActivationFunctionType

    nc = bass.Bass()

    xin = nc.dram_tensor("xin", [C, T], f32, kind="ExternalInput")
    wqkT = nc.dram_tensor("wqkT", [C, 1024], f32r, kind="ExternalInput")
    wvT = nc.dram_tensor("wvT", [C, C], f32r, kind="ExternalInput")
    wpT = nc.dram_tensor("wpT", [C, C], f32r, kind="ExternalInput")
    bqk = nc.dram_tensor("bqk", [1024], f32, kind="ExternalInput")
    bv = nc.dram_tensor("bv", [C], f32, kind="ExternalInput")
    bp = nc.dram_tensor("bp", [C], f32, kind="ExternalInput")
    gam = nc.dram_tensor("gam", [C], f32, kind="ExternalInput")
    bet = nc.dram_tensor("bet", [C], f32, kind="ExternalInput")
    onesc = nc.dram_tensor("onesc", [128, 8], f32r, kind="ExternalInput")
    maskA = nc.dram_tensor("maskA", [128, 8], f32r, kind="ExternalInput")
    maskB = nc.dram_tensor("maskB", [8, 128], f32r, kind="ExternalInput")
    outd = nc.dram_tensor("outd", [C, T], f32, kind="ExternalOutput")

    with tile.TileContext(nc) as tc:
        with tc.tile_pool(name="const", bufs=1) as const, \
             tc.tile_pool(name="big", bufs=1) as big, \
             tc.tile_pool(name="qpp", bufs=2) as qpp, \
             tc.tile_pool(name="kpp", bufs=4) as kpp, \
             tc.tile_pool(name="esp", bufs=6) as esp, \
             tc.tile_pool(name="accp", bufs=1) as accp, \
             tc.tile_pool(name="zp", bufs=3) as zp, \
             tc.tile_pool(name="gn", bufs=2) as gn, \
             tc.tile_pool(name="ps", bufs=2, space="PSUM") as ps, \
             tc.tile_pool(name="dram", bufs=2, space="DRAM") as dram:

            # ---- loads.  x + groupnorm consts on the Sync queue (critical
            # path); weights and the rest issued from the GpSimd queue so they
            # don't delay groupnorm. ----
            # tiny critical-path constants first so they are not queued
            # behind the bulk x/weight traffic
            gam_t = const.tile([128, 4], f32)
            nc.sync.dma_start(out=gam_t, in_=gam.rearrange("(ci p) -> p ci", p=128))
            bet_t = const.tile([128, 4], f32)
            nc.sync.dma_start(out=bet_t, in_=bet.rearrange("(ci p) -> p ci", p=128))
            mA = const.tile([128, 8], f32r)
            nc.sync.dma_start(out=mA, in_=maskA[:, :])
            mB = const.tile([8, 128], f32r)
            nc.sync.dma_start(out=mB, in_=maskB[:, :])
            eps_t = const.tile([128, 1], f32)
            nc.vector.memset(eps_t, EPS)

            xt = [big.tile([128, 1024], f32, tag=f"x{ci}", name=f"x{ci}") for ci in range(4)]
            xr = xin.rearrange("(ci p) t -> p ci t", p=128)
            for ci in range(4):
                eng = nc.sync if ci % 2 == 0 else nc.scalar
                eng.dma_start(out=xt[ci], in_=xr[:, ci, :])
            wvT_t = const.tile([128, 4, 512], f32r)
            nc.gpsimd.dma_start(out=wvT_t, in_=wvT.rearrange("(ci p) o -> p ci o", p=128))
            bqk_t = const.tile([128, 8], f32)
            nc.gpsimd.dma_start(out=bqk_t, in_=bqk.rearrange("(oi p) -> p oi", p=128))
            bv_b = const.tile([128, 512], f32)
            nc.gpsimd.dma_start(out=bv_b, in_=bass.AP(
                tensor=bv, offset=0, ap=[[0, 128], [1, 512]]))
            bp_t = const.tile([128, 4], f32)
            nc.gpsimd.dma_start(out=bp_t, in_=bp.rearrange("(ci p) -> p ci", p=128))
            wqkT_t = const.tile([128, 4, 1024], f32r)
            wqr = wqkT.rearrange("(ci p) o -> p ci o", p=128)
            for ci in range(4):
                eng = nc.sync if ci < 2 else nc.scalar
                eng.dma_start(out=wqkT_t[:, ci, :], in_=wqr[:, ci, :])
            wpT_t = const.tile([128, 4, 512], f32r)
            nc.gpsimd.dma_start(out=wpT_t, in_=wpT.rearrange("(ci p) o -> p ci o", p=128))

            # ---- GroupNorm ----
            # per-channel mean/var over t (bn_stats in 2 chunks of 512)
            chmv = gn.tile([128, 4, 2], f32)
            for ci in range(4):
                st = gn.tile([128, 2, 6], f32, tag="st")
                xv = xt[ci].rearrange("p (n f) -> p n f", f=512)
                for sub in range(2):
                    nc.vector.bn_stats(out=st[:, sub, :], in_=xv[:, sub, :])
                nc.vector.bn_aggr(out=chmv[:, ci, :], in_=st)
            # per-channel [mean, var+mean^2] as f32r for the mask matmul
            s2ch = gn.tile([128, 4, 2], f32r)
            nc.vector.tensor_copy(out=s2ch[:, :, 0], in_=chmv[:, :, 0])
            t1 = gn.tile([128, 4], f32)
            nc.vector.tensor_mul(out=t1, in0=chmv[:, :, 0], in1=chmv[:, :, 0])
            nc.vector.tensor_add(out=s2ch[:, :, 1], in0=t1, in1=chmv[:, :, 1])
            # group sums: [8 groups-in-tile, (ci, k)]
            pg = ps.tile([128, 1024], f32, tag="s", name="pgn")
            nc.tensor.matmul(pg[0:8, 0:8], lhsT=mA,
                             rhs=s2ch.rearrange("p a b -> p (a b)"),
                             start=True, stop=True)
            gf = gn.tile([8, 4, 2], f32r)
            mg = gn.tile([8, 4], f32)
            nc.vector.tensor_scalar_mul(out=mg, in0=pg[0:8, 0:8].rearrange(
                "g (a b) -> g a b", b=2)[:, :, 0], scalar1=1.0 / GS)
            vg = gn.tile([8, 4], f32)
            nc.vector.tensor_scalar_mul(out=vg, in0=pg[0:8, 0:8].rearrange(
                "g (a b) -> g a b", b=2)[:, :, 1], scalar1=1.0 / GS)
            m2 = gn.tile([8, 4], f32)
            nc.vector.tensor_mul(out=m2, in0=mg, in1=mg)
            nc.vector.tensor_sub(out=vg, in0=vg, in1=m2)
            # rstd = 1/sqrt(vg + eps)
            nc.scalar.activation(out=vg, in_=vg, func=Act.Sqrt, bias=eps_t[:8], scale=1.0)
            nc.vector.reciprocal(out=vg, in_=vg)
            nc.vector.tensor_copy(out=gf[:, :, 0], in_=mg)
            nc.vector.tensor_copy(out=gf[:, :, 1], in_=vg)
            # broadcast group stats back to channels: [128, (ci, k)]
            pc = ps.tile([128, 1024], f32, tag="s", name="pgc")
            nc.tensor.matmul(pc[:, 0:8], lhsT=mB,
                             rhs=gf.rearrange("g a b -> g (a b)"),
                             start=True, stop=True)
            chms = pc[:, 0:8].rearrange("p (a b) -> p a b", b=2)
            # per-channel scale/shift  [128, 4]
            scl = gn.tile([128, 4], f32)
            nc.vector.tensor_mul(out=scl, in0=gam_t, in1=chms[:, :, 1])
            sht = gn.tile([128, 4], f32)
            nc.vector.tensor_mul(out=sht, in0=scl, in1=chms[:, :, 0])
            nc.vector.tensor_sub(out=sht, in0=bet_t, in1=sht)
            # h = x * scl + sht   (f32r, feeds matmuls)
            ht = [big.tile([128, 1024], f32r, tag=f"h{ci}", name=f"h{ci}") for ci in range(4)]
            for ci in range(4):
                nc.vector.tensor_scalar(out=ht[ci], in0=xt[ci],
                                        scalar1=scl[:, ci:ci + 1], scalar2=sht[:, ci:ci + 1],
                                        op0=Alu.mult, op1=Alu.add)

            # ---- vT[t, o_v] first (needs only h), so attention can start as
            # soon as the first qkv pair lands ----
            vTa = big.tile([128, 8, 8, 65], f32r)  # [t_part, ti, hd, ch+1]
            for tp in range(4):
                for half in range(2):
                    nc.gpsimd.dma_start(
                        out=vTa[:, 2 * tp + half, :, 64:65].rearrange("p h k -> p (h k)"),
                        in_=onesc[:, :])
                pv = ps.tile([128, 1024], f32, tag="s", name=f"pv{tp}")
                for ci in range(4):
                    for half in range(2):
                        ti = 2 * tp + half
                        nc.tensor.matmul(pv[:, half * 512:(half + 1) * 512],
                                         lhsT=ht[ci][:, ti * 128:(ti + 1) * 128],
                                         rhs=wvT_t[:, ci, :], start=(ci == 0), stop=(ci == 3))
                nc.vector.tensor_add(
                    out=vTa[:, 2 * tp:2 * tp + 2, :, 0:64],
                    in0=pv.rearrange("p (t h c) -> p t h c", t=2, h=8),
                    in1=bass.AP(tensor=bv_b.tensor, offset=bv_b.offset,
                                ap=[list(bv_b.ap[0])] + [[0, 2]] + [[64, 8], [1, 64]]))

            # ---- q,k for one head pair.  q stays paired [q_A; q_B] on 128
            # partitions; each head's k is zero-padded to the full 128
            # partitions so the score matmul runs with K=128 at the fast PE
            # rate (the zero rows multiply the other head's q to nothing;
            # matmul cost only depends on N). ----
            qp = [None] * 4
            kpad = [None] * 8

            def qkv_pair_thunks(pi):
                """Emit-later pieces of the pair's q/k computation, so they can
                be spread through the previous pair's attention loop."""
                qp[pi] = qpp.tile([128, 1024], f32r, tag="qp", name=f"qp{pi}")
                kpad[2 * pi] = kpp.tile([128, 1024], f32r, tag="kp", name=f"kp{2*pi}")
                kpad[2 * pi + 1] = kpp.tile([128, 1024], f32r, tag="kp", name=f"kp{2*pi+1}")
                nc.gpsimd.memset(kpad[2 * pi][64:128, :].bitcast(f32), 0.0)
                nc.gpsimd.memset(kpad[2 * pi + 1][0:64, :].bitcast(f32), 0.0)
                thunks = []
                state = {}

                def mk_mm(side, ci):
                    def t():
                        oi = side * 4 + pi
                        if ci == 0:
                            state[side] = ps.tile([128, 1024], f32, tag="s",
                                                  name=f"pqk{oi}")
                        pqk = state[side]
                        for ni in range(2):
                            nc.tensor.matmul(pqk[:, ni * 512:(ni + 1) * 512],
                                             lhsT=wqkT_t[:, ci, oi * 128:(oi + 1) * 128],
                                             rhs=ht[ci][:, ni * 512:(ni + 1) * 512],
                                             start=(ci == 0), stop=(ci == 3))
                    return t

                def mk_evac(side):
                    def t():
                        oi = side * 4 + pi
                        pqk = state[side]
                        if side == 0:
                            nc.vector.tensor_scalar_add(out=qp[pi], in0=pqk,
                                                        scalar1=bqk_t[:, oi:oi + 1])
                        else:
                            nc.vector.tensor_scalar_add(
                                out=kpad[2 * pi][0:64, :], in0=pqk[0:64, :],
                                scalar1=bqk_t[0:64, oi:oi + 1])
                            nc.vector.tensor_scalar_add(
                                out=kpad[2 * pi + 1][64:128, :], in0=pqk[64:128, :],
                                scalar1=bqk_t[64:128, oi:oi + 1])
                    return t

                for side in range(2):
                    for ci in range(4):
                        thunks.append(mk_mm(side, ci))
                    thunks.append(mk_evac(side))
                return thunks

            def qkv_pair(pi):
                for t in qkv_pair_thunks(pi):
                    t()

            # ---- attention (one head pair at a time); qkv for the next pair
            # and proj partials for finished pairs are interleaved so the PE
            # fills its slack while ACT (exp) is the bottleneck ----
            at_ = [None] * 4   # a[c(hd-major), t] per pair
            acc = [accp.tile([128, 1024], f32, tag=f"acc{oi}", name=f"acc{oi}")
                   for oi in range(4)]
            pa_all = [None] * 4

            def attn_core(pi, filler=()):
                filler = list(filler)
                pa = [ps.tile([128, 1024], f32, tag="a", name=f"pa{pi}_{i}") for i in range(2)]
                pa_all[pi] = pa
                for si in range(8):
                    if si >= 2:
                        for _ in range(2):
                            if filler:
                                filler.pop(0)()
                    pss = [ps.tile([128, 1024], f32, tag="s", name=f"pss{pi}_{si}_{i}")
                           for i in range(2)]
                    for half in range(2):
                        for ni in range(2):
                            nc.tensor.matmul(
                                pss[half][:, ni * 512:(ni + 1) * 512],
                                lhsT=kpad[2 * pi + half][:, si * 128:(si + 1) * 128],
                                rhs=qp[pi][:, ni * 512:(ni + 1) * 512],
                                start=True, stop=True)
                    ess = []
                    for half in range(2):
                        es = esp.tile([128, 1024], f32r, tag="es")
                        nc.scalar.activation(out=es, in_=pss[half], func=Act.Exp)
                        ess.append(es)
                    for half in range(2):
                        hd = 2 * pi + half
                        for ni in range(2):
                            nc.tensor.matmul(pa[half][0:65, ni * 512:(ni + 1) * 512],
                                             lhsT=vTa[:, si, hd, :],
                                             rhs=ess[half][:, ni * 512:(ni + 1) * 512],
                                             start=(si == 0), stop=(si == 7))
                for t in filler:
                    t()

            def divide(pi):
                at_[pi] = big.tile([128, 1024], f32r, tag="at", name=f"at{pi}")
                pa = pa_all[pi]
                aus, zbs = [], []
                # phase 1: evacuate both heads (releases the psum accumulators)
                for half in range(2):
                    au = zp.tile([65, 1024], f32, tag="au")
                    nc.vector.tensor_copy(out=au, in_=pa[half][0:65, :])
                    aus.append(au)
                # phase 2: reciprocal of Z via [128, 8] reshape (DRAM round trips)
                for half in range(2):
                    eng = nc.sync if half == 0 else nc.gpsimd
                    zd = dram.tile([1, 1024], f32, tag="zd")
                    eng.dma_start(out=zd, in_=aus[half][64:65, :])
                    zq = zp.tile([128, 8], f32, tag="zq")
                    eng.dma_start(out=zq, in_=bass.AP(
                        tensor=zd.tensor, offset=zd.offset, ap=[[8, 128], [1, 8]]))
                    nc.vector.reciprocal(out=zq, in_=zq)
                    zd2 = dram.tile([1, 1024], f32, tag="zd2")
                    eng.dma_start(
                        out=bass.AP(tensor=zd2.tensor, offset=zd2.offset,
                                    ap=[[8, 128], [1, 8]]), in_=zq)
                    zb = zp.tile([64, 1024], f32, tag="zb")
                    eng.dma_start(out=zb, in_=bass.AP(
                        tensor=zd2.tensor, offset=zd2.offset, ap=[[0, 64], [1, 1024]]))
                    zbs.append(zb)
                # phase 3: normalize
                for half in range(2):
                    hd = 2 * pi + half
                    nc.vector.tensor_tensor(
                        out=at_[pi][(hd % 2) * 64:(hd % 2) * 64 + 64, :],
                        in0=aus[half][0:64, :], in1=zbs[half], op=Alu.mult)

            def proj_part(pi, tag="a"):
                # contribution of channel block pi to every output block
                for oi in range(4):
                    pp = ps.tile([128, 1024], f32, tag=tag, name=f"pp{pi}_{oi}")
                    for ni in range(2):
                        nc.tensor.matmul(pp[:, ni * 512:(ni + 1) * 512],
                                         lhsT=wpT_t[:, pi, oi * 128:(oi + 1) * 128],
                                         rhs=at_[pi][:, ni * 512:(ni + 1) * 512],
                                         start=True, stop=True)
                    if pi == 0:
                        nc.vector.scalar_tensor_tensor(
                            out=acc[oi], in0=pp, scalar=bp_t[:, oi:oi + 1],
                            in1=xt[oi], op0=Alu.add, op1=Alu.add)
                    else:
                        nc.vector.tensor_add(out=acc[oi], in0=acc[oi], in1=pp)
                    if pi == 3:
                        eng = nc.sync if oi % 2 == 0 else nc.gpsimd
                        eng.dma_start(
                            out=outd.rearrange("(ci p) t -> p ci t", p=128)[:, oi, :],
                            in_=acc[oi])

            qkv_pair(0)
            attn_core(0)
            qkv_pair(1)
            divide(0)
            attn_core(1)
            qkv_pair(2)
            divide(1)
            proj_part(0)
            attn_core(2)
            qkv_pair(3)
            divide(2)
            proj_part(1)
            attn_core(3)
            proj_part(2, tag="s")
            divide(3)
            proj_part(3, tag="s")

    _split_multi_waits(nc)
    return nc


def _prep_host(x, gamma, beta, w_qkv, b_qkv, w_proj, b_proj):
    """Host-side weight permutation/pre-scaling + per-core input maps."""
    x = np.ascontiguousarray(x, dtype=np.float32).reshape(B, C, T)
    scale2 = 1.0 / np.sqrt(CH)  # folded into q (exact: 0.125 is a power of two)

    w = np.asarray(w_qkv, dtype=np.float32).reshape(H, 3, CH, C)
    bq = np.asarray(b_qkv, dtype=np.float32).reshape(H, 3, CH)
    wq = w[:, 0] * scale2          # [hd, 64, c]
    wk = w[:, 1]
    wv = w[:, 2]
    # o-block order: 4 q-blocks (one per head pair: [q_{2i}; q_{2i+1}]), 4 k-blocks
    qcols = wq.reshape(4, 128, C).transpose(2, 0, 1)            # [c, pi, 128]
    kcols = wk.reshape(4, 128, C).transpose(2, 0, 1)
    wqkT_host = np.ascontiguousarray(
        np.concatenate([qcols, kcols], axis=1).reshape(C, 1024))
    bqk_host = np.ascontiguousarray(np.concatenate(
        [(bq[:, 0] * scale2).reshape(4, 128), bq[:, 1].reshape(4, 128)], axis=0
    ).reshape(1024))
    wvT_host = np.ascontiguousarray(wv.transpose(2, 0, 1).reshape(C, C))
    bv_host = np.ascontiguousarray(bq[:, 2].reshape(C))
    wpT_host = np.ascontiguousarray(np.asarray(w_proj, dtype=np.float32).T)
    ones_host = np.ones((128, 8), np.float32)
    maskA_host = np.zeros((128, 8), np.float32)
    for p in range(128):
        maskA_host[p, p // 16] = 1.0
    maskB_host = np.ascontiguousarray(maskA_host.T)

    common = {
        "wqkT": wqkT_host, "wvT": wvT_host, "wpT": wpT_host,
        "bqk": bqk_host, "bv": bv_host,
        "bp": np.ascontiguousarray(np.asarray(b_proj, np.float32)),
        "gam": np.ascontiguousarray(np.asarray(gamma, np.float32)),
        "bet": np.ascontiguousarray(np.asarray(beta, np.float32)),
        "onesc": ones_host, "maskA": maskA_host, "maskB": maskB_host,
    }
    return [dict(common, xin=np.ascontiguousarray(x[b])) for b in range(B)]


def kernel(x, gamma, beta, w_qkv, b_qkv, w_proj, b_proj):
    from concourse.bass_utils import run_bass_kernel_spmd

    # _patch_ldw_opt()  # caused a runtime deadlock; see notes
    if "nc" not in _CACHE:
        _CACHE["nc"] = _build_nc()
    nc = _CACHE["nc"]

    in_maps = _prep_host(x, gamma, beta, w_qkv, b_qkv, w_proj, b_proj)
    kwargs = {}
    if TRACE:
        _install_ntff_hook()
        kwargs["trace"] = True
    res = run_bass_kernel_spmd(nc, in_maps, core_ids=list(range(NCORES)), **kwargs)
    if TRACE:
        _CACHE["last_result"] = res
    out = np.stack([r["outd"] for r in res.results], axis=0)
    return out.reshape(B, C, HW, HW)
